# revision 4
# baseline (speedup 1.0000x reference)
"""CNN-LSTM kernel for Trainium2, 8-core data parallel.

Model: Conv1d(1024->768->384->192, k=3, pad=1, ReLU) x3 -> MaxPool1d(2)
       -> 2-layer LSTM(H=512) -> masked mean over valid steps -> FC head.

Sharding: batch 64 split 8 ways (8 samples per core); weights replicated.

Layouts (per core, all channel-major so nothing needs an on-device transpose):
  x / conv activations: [C_part, batch=8, time(+2 pad)]
  LSTM gate psum:       [128, 16 blocks, 8]  (blocks host-permuted:
                        [i0,f0,o0, i1,f1,o1, ..., g0,g1,g2,g3])
  h state:              [128, 4 hchunk, 8] (bf16 ping-pong)
  masked mean: acc += h_t * (mask[t,b]/a[b]) each step.
"""

from contextlib import ExitStack

import numpy as np
import ml_dtypes

import concourse.bass as bass
import concourse.mybir as mybir
import concourse.tile as tile
from concourse import bacc
from concourse.bass import ds
from concourse.bass_utils import run_bass_kernel_spmd

BF = ml_dtypes.bfloat16
bf16 = mybir.dt.bfloat16
f32 = mybir.dt.float32
ET = mybir.EngineType
f8e4 = mybir.dt.float8e4
AFT = mybir.ActivationFunctionType

N_CORES = 8
B, BC = 64, 8
CIN, T, T2, H, G = 1024, 512, 256, 512, 2048
C1, C2, C3 = 768, 384, 192
UNROLL = 32
FP8 = True
ACT_SCALE = 8.0


# gate block order (chain-consumption order): [g0..g3 | i0,f0,i1,f1,i2,f2,
# i3,f3 | o0..o3].  psum group A = blocks 0:4 (tanh g), B = 4:12 (sigmoid
# i/f), C = 12:16 (sigmoid o).  The burst fills A, then B, then C so the
# tail's ACT ops start while the burst is still issuing, and sigmoid(o)
# overlaps the DVE c-update.
def gate_perm():
    g = lambda base, m: base + 128 * m + np.arange(128)
    blocks = [g(1024, m) for m in range(4)]          # g_m
    for m in range(4):
        blocks.append(g(0, m))                       # i_m
        blocks.append(g(512, m))                     # f_m
    blocks += [g(1536, m) for m in range(4)]         # o_m
    return np.concatenate(blocks)


GATE_PERM = gate_perm()

TAP_SPECS = {
    "y1": ([C1, BC, T], bf16), "y2": ([C2, BC, T], bf16),
    "y3": ([C3, BC, T], bf16), "pl": ([C3, BC, T2], bf16),
    "xp0": ([G, BC, T2], bf16), "h0": ([H, BC, T2], bf16),
    "xp1": ([G, BC, T2], bf16), "macc": ([H, BC], f32),
}


def build_kernel(taps=()):
    nc = bacc.Bacc("TRN2", target_bir_lowering=False, debug=False,
                   num_devices=N_CORES)
    din = {}

    def inp(name, shape, dt=bf16):
        din[name] = nc.dram_tensor(name, shape, dt, kind="ExternalInput").ap()

    inp("xb", [CIN, BC, T + 2])                 # padded time, cols 0,T+1 zero
    inp("w1", [CIN, 3, C1])
    inp("w2", [C1, 3, C2])
    inp("w3", [C2, 3, C3])
    inp("cb1", [C1], f32)
    inp("cb2", [C2], f32)
    inp("cb3", [C3], f32)
    whh_dt = f8e4 if FP8 else bf16
    inp("wih0", [C3, G])                        # gate cols permuted
    inp("whh0", [H, G], whh_dt)
    inp("bg0", [G], f32)
    inp("wih1", [H, G])
    inp("whh1", [H, G], whh_dt)
    inp("bg1", [G], f32)
    inp("fw1", [H, 256])
    inp("fw2", [256, 64])
    inp("fw3", [64, 8])
    inp("fw4", [8, 1])
    inp("fb1", [256], f32)
    inp("fb2", [64], f32)
    inp("fb3", [8], f32)
    inp("fb4", [1], f32)
    inp("mk", [BC, T2], f32)                    # mask[t,b]/a[b], b-major
    inp("ident", [128, 128])                    # identity for xp->psum fold

    out = nc.dram_tensor("out", [BC], f32, kind="ExternalOutput").ap()
    dtap = {}
    for tp in taps:
        shp, dt = TAP_SPECS[tp]
        dtap[tp] = nc.dram_tensor("tap_" + tp, shp, dt,
                                  kind="ExternalOutput").ap()

    with tile.TileContext(nc) as tc:
        _body(nc, tc, din, out, dtap)
    nc.compile()
    return nc


def _conv_layer(nc, psum, wsb, bsb, src, dst, tap):
    """src: list of [128, BC, T+2] padded tiles; dst: list of co-chunk tiles,
    padded [p, BC, T+2] (written at [:, :, 1:T+1]) or [p, BC, T].
    wsb: per ci-chunk [128, 3, co] tiles. bsb: [128, n_co] tile."""
    nci = len(src)
    pads = dst[0].shape[2] == T + 2
    for j, dtile in enumerate(dst):
        mj = dtile.shape[0]
        for b in range(BC):
            ps = psum.tile([128, T], f32, tag="pscv", name="pscv")
            n = 0
            for c in range(nci):
                for k in range(3):
                    nc.tensor.matmul(
                        ps[:mj, :],
                        wsb[c][:, k, 128 * j:128 * j + mj],
                        src[c][:, b, k:k + T],
                        start=(n == 0), stop=(n == 3 * nci - 1))
                    n += 1
            od = dtile[:, b, 1:T + 1] if pads else dtile[:, b, :]
            nc.scalar.activation(out=od, in_=ps[:mj, :], func=AFT.Relu,
                                 bias=bsb[:mj, j:j + 1], scale=1.0)
        if tap is not None:
            nc.sync.dma_start(
                out=tap[128 * j:128 * j + mj, :, :],
                in_=dtile[:, :, 1:T + 1] if pads else dtile[:, :, :])


def _proj(nc, psum, wsb, bias_sb, rhs_slices, xp, nk, tap):
    """xp[:, n, :, :] = sum_k wsb[k][:, nblk].T @ rhs_k + bias.
    wsb: nk tiles [<=128, G]; rhs_slices(k, s) -> [<=128, 2, T2] AP;
    xp: [128, 16, BC, T2] bf16; bias_sb [128, 16] f32."""
    for n in range(16):
        for s in range(BC // 2):
            ps = psum.tile([128, 2, T2], f32, tag="pscv", name="pscv")
            for k in range(nk):
                nc.tensor.matmul(ps[:, :, :],
                                 wsb[k][:, 128 * n:128 * n + 128],
                                 rhs_slices(k, s),
                                 start=(k == 0), stop=(k == nk - 1))
            dst = xp[:, n, 2 * s:2 * s + 2, :]
            if (n + s) % 2 == 0:
                nc.scalar.activation(out=dst, in_=ps, func=AFT.Identity,
                                     bias=bias_sb[:, n:n + 1], scale=1.0)
            else:
                nc.vector.tensor_scalar_add(out=dst, in0=ps,
                                            scalar1=bias_sb[:, n:n + 1])
        if tap is not None:
            nc.sync.dma_start(out=tap[128 * n:128 * (n + 1), :, :],
                              in_=xp[:, n, :, :])


def _lstm_fused(nc, tc, xp, whh0_sb, whh1_sb, wih1_sb, ident, bg1, msk4,
                macc, prec, pwin, psum, psrec, act_scale):
    """Both LSTM layers, L1 lagging L0 by one 32-step window.
    Gate blocks are host-permuted to [g|if|o] so the burst fills three psum
    groups in chain order: tanh(g) and sigmoid(i,f) run on ACT while the
    burst is still issuing, sigmoid(o) overlaps the DVE c-update.
    L0's h is written straight into h0win (no copy); the next step's burst
    streams h0win[:, k, :, u-1].  hlast carries h across window boundaries."""
    W = UNROLL
    hb1 = [prec.tile([128, 4, BC], bf16, tag=f"hb1_{p}", name=f"hb1_{p}") for p in (0, 1)]
    c0 = prec.tile([128, 4, BC], f32, tag="c0", name="c0")
    c1 = prec.tile([128, 4, BC], f32, tag="c1", name="c1")
    hlast = prec.tile([128, 4, BC], bf16, tag="hlast", name="hlast")
    h0win = pwin.tile([128, 4, BC, W], bf16, tag="h0win", name="h0win")
    xp1ring = pwin.tile([128, 16, BC, W], bf16, tag="xp1ring", name="xp1ring")
    for tl in (hb1[0], c0, c1, hlast):
        nc.vector.memset(tl, 0.0)
    inv = 1.0 / act_scale

    def tail(layer, ps3, c_st, hdst, t, u):
        psA, psB, psC = ps3
        tg = prec.tile([128, 4, BC], f32, tag=f"tg{layer}", name=f"tg{layer}")
        sif = prec.tile([128, 8, BC], f32, tag=f"sif{layer}", name=f"sif{layer}")
        so = prec.tile([128, 4, BC], f32, tag=f"so{layer}", name=f"so{layer}")
        nc.scalar.activation(out=tg, in_=psA, func=AFT.Tanh, scale=inv)
        nc.scalar.activation(out=sif, in_=psB, func=AFT.Sigmoid, scale=inv)
        nc.scalar.activation(out=so, in_=psC, func=AFT.Sigmoid, scale=inv)
        sifv = sif.rearrange("p (m q) b -> p m q b", q=2)
        t1 = prec.tile([128, 4, BC], f32, tag=f"t1{layer}", name=f"t1{layer}")
        t2 = prec.tile([128, 4, BC], f32, tag=f"t2{layer}", name=f"t2{layer}")
        nc.gpsimd.tensor_mul(t1, sifv[:, :, 1, :], c_st)
        nc.vector.tensor_mul(t2, sifv[:, :, 0, :], tg)
        nc.vector.tensor_add(c_st, t1, t2)
        tc_ = prec.tile([128, 4, BC], f32, tag=f"tc{layer}", name=f"tc{layer}")
        nc.scalar.activation(out=tc_, in_=c_st, func=AFT.Tanh)
        nc.vector.tensor_mul(hdst, so, tc_)
        if layer == 0:
            if u == W - 1:
                nc.gpsimd.tensor_copy(out=hlast, in_=hdst)
        else:
            tm = prec.tile([128, 4, BC], f32, tag="tm", name="tm")
            nc.gpsimd.tensor_mul(tm, hdst, msk4[:, :, :, ds(t, 1)])
            nc.gpsimd.tensor_add(macc, macc, tm)

    GRP = ((0, 4), (4, 12), (12, 16))

    def burst(ps3, id_rhs, whh_sb, hcol):
        for (j0, j1), ps in zip(GRP, ps3):
            nc.tensor.matmul(ps[:, :, :], ident, id_rhs(j0, j1),
                             start=True, stop=False, skip_group_check=True)
            for j in range(j0, j1):
                for k in range(4):
                    nc.tensor.matmul(ps[:, j - j0, :],
                                     whh_sb[k][:, 128 * j:128 * j + 128],
                                     hcol(k),
                                     start=False,
                                     stop=(j == j1 - 1 and k == 3),
                                     skip_group_check=True)

    def ps3_new(layer):
        ps = psrec.tile([128, 16, BC], f32, tag=f"ps{layer}",
                        name=f"ps{layer}")
        return tuple(ps[:, j0:j1, :] for j0, j1 in GRP)

    def step_l0(t, u):
        ps3 = ps3_new(0)
        if u == 0:
            hcol = lambda k: hlast[:, k, :]
        else:
            hcol = lambda k: h0win[:, k, :, u - 1]
        burst(ps3, lambda j0, j1: xp[:, j0:j1, :, ds(t, 1)], whh0_sb, hcol)
        tail(0, ps3, c0, h0win[:, :, :, u], t, u)

    def step_l1(t, u, par):
        ps3 = ps3_new(1)
        burst(ps3, lambda j0, j1: xp1ring[:, j0:j1, :, u], whh1_sb,
              lambda k: hb1[par][:, k, :])
        tail(1, ps3, c1, hb1[1 - par], t, u)

    def xp1batch():
        for n in range(16):
            ps = psum.tile([128, BC, W], f32, tag="pscv", name="pscv")
            for k in range(4):
                nc.tensor.matmul(ps[:, :, :],
                                 wih1_sb[k][:, 128 * n:128 * n + 128],
                                 h0win[:, k, :, :],
                                 start=(k == 0), stop=(k == 3))
            dst = xp1ring[:, n, :, :]
            if n % 2 == 0:
                nc.scalar.activation(out=dst, in_=ps, func=AFT.Identity,
                                     bias=bg1[:, n:n + 1], scale=1.0)
            else:
                nc.vector.tensor_scalar_add(out=dst, in0=ps,
                                            scalar1=bg1[:, n:n + 1])

    # prologue window: L0 steps 0..W-1, then first xp1 batch
    for u in range(W):
        step_l0(u, u)
    xp1batch()
    # main: windows with L1 lagging one window
    with tc.For_i(W, T2, W, staggered_reset=True,
                  hint_engines=(ET.PE, ET.Activation, ET.DVE, ET.Pool)) as iv:
        for u in range(W):
            step_l0(iv + u, u)
            step_l1(iv + u - W, u, u & 1)
        xp1batch()
    # epilogue: last window of L1
    for u in range(W):
        step_l1(T2 - W + u, u, u & 1)


def _body(nc, tc, din, out, dtap):
    with ExitStack() as top:
        pmisc = top.enter_context(tc.tile_pool(name="pmisc", bufs=1))
        psum = top.enter_context(tc.tile_pool(name="psum", bufs=2, space="PSUM"))
        psrec = top.enter_context(tc.tile_pool(name="psrec", bufs=3, space="PSUM"))
        prec = top.enter_context(tc.tile_pool(name="prec", bufs=2))
        ppl = top.enter_context(tc.tile_pool(name="ppl", bufs=1))

        # small persistent constants
        cb1 = pmisc.tile([128, 6], f32)
        cb2 = pmisc.tile([128, 3], f32)
        cb3 = pmisc.tile([128, 2], f32)
        nc.sync.dma_start(out=cb1, in_=din["cb1"].rearrange("(j p) -> p j", p=128))
        nc.sync.dma_start(out=cb2, in_=din["cb2"].rearrange("(j p) -> p j", p=128))
        nc.vector.memset(cb3, 0.0)
        nc.sync.dma_start(out=cb3[:, 0:1], in_=din["cb3"][0:128])
        nc.sync.dma_start(out=cb3[0:64, 1:2], in_=din["cb3"][128:192])
        bg0 = pmisc.tile([128, 16], f32)
        bg1 = pmisc.tile([128, 16], f32)
        nc.sync.dma_start(out=bg0, in_=din["bg0"].rearrange("(j p) -> p j", p=128))
        nc.sync.dma_start(out=bg1, in_=din["bg1"].rearrange("(j p) -> p j", p=128))

        # pool stack: ppl(top-level) > py2 > py1 > {px,pw1}, {pw2}, ...
        st2 = ExitStack()   # py2: closes after conv3
        st1 = ExitStack()   # py1: closes after conv2
        st0 = ExitStack()   # px + pw1: closes after conv1
        py2 = st2.enter_context(tc.tile_pool(name="py2", bufs=1))
        py1 = st1.enter_context(tc.tile_pool(name="py1", bufs=1))
        px = st0.enter_context(tc.tile_pool(name="px", bufs=1))
        pw1 = st0.enter_context(tc.tile_pool(name="pw1", bufs=1))

        # ---------------- conv1 ----------------
        y1 = [py1.tile([128, BC, T + 2], bf16, tag=f"y1_{j}", name=f"y1_{j}")
              for j in range(6)]
        for yt in y1:
            nc.gpsimd.memset(yt[:, :, 0:1], 0.0)
            nc.gpsimd.memset(yt[:, :, T + 1:T + 2], 0.0)
        xsb = [px.tile([128, BC, T + 2], bf16, tag=f"x_{c}", name=f"x_{c}")
               for c in range(8)]
        w1sb = [pw1.tile([128, 3, C1], bf16, tag=f"w1_{c}", name=f"w1_{c}")
                for c in range(8)]
        for c in range(8):
            nc.sync.dma_start(out=xsb[c], in_=din["xb"][128 * c:128 * (c + 1)])
            nc.sync.dma_start(out=w1sb[c], in_=din["w1"][128 * c:128 * (c + 1)])
        _conv_layer(nc, psum, w1sb, cb1, xsb, y1, dtap.get("y1"))
        st0.close()

        # ---------------- conv2 ----------------
        stw2 = ExitStack()
        pw2 = stw2.enter_context(tc.tile_pool(name="pw2", bufs=1))
        y2 = [py2.tile([128, BC, T + 2], bf16, tag=f"y2_{j}", name=f"y2_{j}")
              for j in range(3)]
        for yt in y2:
            nc.gpsimd.memset(yt[:, :, 0:1], 0.0)
            nc.gpsimd.memset(yt[:, :, T + 1:T + 2], 0.0)
        w2sb = [pw2.tile([128, 3, C2], bf16, tag=f"w2_{c}", name=f"w2_{c}")
                for c in range(6)]
        for c in range(6):
            nc.sync.dma_start(out=w2sb[c], in_=din["w2"][128 * c:128 * (c + 1)])
        _conv_layer(nc, psum, w2sb, cb2, y1, y2, dtap.get("y2"))
        stw2.close()
        st1.close()

        # ---------------- conv3 + maxpool ----------------
        st3 = ExitStack()
        py3 = st3.enter_context(tc.tile_pool(name="py3", bufs=1))
        pw3 = st3.enter_context(tc.tile_pool(name="pw3", bufs=1))
        y3 = [py3.tile([128, BC, T], bf16, tag="y3_0", name="y3_0"),
              py3.tile([64, BC, T], bf16, tag="y3_1", name="y3_1")]
        w3sb = [pw3.tile([128, 3, C3], bf16, tag=f"w3_{c}", name=f"w3_{c}")
                for c in range(3)]
        for c in range(3):
            nc.sync.dma_start(out=w3sb[c], in_=din["w3"][128 * c:128 * (c + 1)])
        _conv_layer(nc, psum, w3sb, cb3, y2, y3, dtap.get("y3"))

        pl = [ppl.tile([128, BC, T2], bf16, tag="pl_0", name="pl_0"),
              ppl.tile([64, BC, T2], bf16, tag="pl_1", name="pl_1")]
        for j in range(2):
            yr = y3[j].rearrange("p b (t k) -> p b t k", k=2)
            nc.vector.tensor_max(pl[j], yr[:, :, :, 0], yr[:, :, :, 1])
            if "pl" in dtap:
                nc.sync.dma_start(
                    out=dtap["pl"][128 * j:128 * j + pl[j].shape[0]], in_=pl[j])
        st3.close()
        st2.close()

        # ---------------- LSTM weights + xp0 ----------------
        plstm = top.enter_context(tc.tile_pool(name="plstm", bufs=1))
        pxp = top.enter_context(tc.tile_pool(name="pxp", bufs=1))

        wih0 = [plstm.tile([128, G], bf16, tag="wih0_0", name="wih0_0"),
                plstm.tile([64, G], bf16, tag="wih0_1", name="wih0_1")]
        nc.sync.dma_start(out=wih0[0], in_=din["wih0"][0:128])
        nc.sync.dma_start(out=wih0[1], in_=din["wih0"][128:192])
        whh_dt = f8e4 if FP8 else bf16
        whh0 = [plstm.tile([128, G], whh_dt, tag=f"whh0_{k}", name=f"whh0_{k}")
                for k in range(4)]
        wih1 = [plstm.tile([128, G], bf16, tag=f"wih1_{k}", name=f"wih1_{k}")
                for k in range(4)]
        whh1 = [plstm.tile([128, G], whh_dt, tag=f"whh1_{k}", name=f"whh1_{k}")
                for k in range(4)]
        for k in range(4):
            nc.sync.dma_start(out=whh0[k], in_=din["whh0"][128 * k:128 * (k + 1)])
            nc.sync.dma_start(out=wih1[k], in_=din["wih1"][128 * k:128 * (k + 1)])
            nc.sync.dma_start(out=whh1[k], in_=din["whh1"][128 * k:128 * (k + 1)])

        ident = plstm.tile([128, 128], bf16, tag="ident", name="ident")
        nc.sync.dma_start(out=ident, in_=din["ident"])
        msk4 = pxp.tile([128, 4, BC, T2], f32, tag="msk4", name="msk4")
        mk_ap = din["mk"]
        nc.sync.dma_start(out=msk4[:, 0, :, :], in_=bass.AP(
            tensor=mk_ap.tensor, offset=mk_ap.offset,
            ap=[[0, 128]] + list(mk_ap.ap)))
        nc.vector.tensor_copy(out=msk4[:, 1, :, :], in_=msk4[:, 0, :, :])
        nc.vector.tensor_copy(out=msk4[:, 2:4, :, :], in_=msk4[:, 0:2, :, :])
        xp = pxp.tile([128, 16, BC, T2], bf16, tag="xp", name="xp")

        _proj(nc, psum, wih0, bg0,
              lambda k, s: pl[k][:, 2 * s:2 * s + 2, :], xp, 2,
              dtap.get("xp0"))

        # ---------------- fused 2-layer LSTM ----------------
        macc = pmisc.tile([128, 4, BC], f32)
        nc.vector.memset(macc, 0.0)
        _lstm_fused(nc, tc, xp, whh0, whh1, wih1, ident, bg1, msk4, macc,
                    prec, pxp, psum, psrec, ACT_SCALE if FP8 else 1.0)
        if "macc" in dtap:
            for m in range(4):
                nc.sync.dma_start(out=dtap["macc"][128 * m:128 * (m + 1)],
                                  in_=macc[:, m, :])

        # ---------------- FC head ----------------
        fw1 = plstm.tile([128, 4, 256], bf16, tag="fw1", name="fw1")
        fw2 = plstm.tile([128, 2, 64], bf16, tag="fw2", name="fw2")
        fw3 = plstm.tile([64, 8], bf16, tag="fw3", name="fw3")
        fw4 = plstm.tile([8, 1], bf16, tag="fw4", name="fw4")
        nc.sync.dma_start(out=fw1,
                          in_=din["fw1"].rearrange("(k p) m -> p k m", p=128))
        nc.sync.dma_start(out=fw2,
                          in_=din["fw2"].rearrange("(k p) m -> p k m", p=128))
        nc.sync.dma_start(out=fw3, in_=din["fw3"])
        nc.sync.dma_start(out=fw4, in_=din["fw4"])
        fb1 = pmisc.tile([128, 2], f32)
        fb2 = pmisc.tile([64, 1], f32)
        fb3 = pmisc.tile([8, 1], f32)
        fb4 = pmisc.tile([1, 1], f32)
        nc.sync.dma_start(out=fb1,
                          in_=din["fb1"].rearrange("(j p) -> p j", p=128))
        nc.sync.dma_start(out=fb2, in_=din["fb2"])
        nc.sync.dma_start(out=fb3, in_=din["fb3"])
        nc.sync.dma_start(out=fb4, in_=din["fb4"])

        maccb = prec.tile([128, 4, BC], bf16, tag="maccb", name="maccb")
        nc.vector.tensor_copy(out=maccb, in_=macc)
        z1 = prec.tile([128, 2, BC], bf16, tag="z1", name="z1")
        for mj in range(2):
            ps = psrec.tile([128, BC], f32, tag="ps0", name="psfc")
            for k in range(4):
                nc.tensor.matmul(ps, fw1[:, k, 128 * mj:128 * mj + 128],
                                 maccb[:, k, :], start=(k == 0),
                                 stop=(k == 3))
            nc.scalar.activation(out=z1[:, mj, :], in_=ps, func=AFT.Relu,
                                 bias=fb1[:, mj:mj + 1], scale=1.0)
        z2 = prec.tile([64, BC], bf16, tag="z2", name="z2")
        ps2 = psrec.tile([64, BC], f32, tag="ps0", name="psfc")
        for k in range(2):
            nc.tensor.matmul(ps2, fw2[:, k, 0:64], z1[:, k, :],
                             start=(k == 0), stop=(k == 1))
        nc.scalar.activation(out=z2, in_=ps2, func=AFT.Relu,
                             bias=fb2, scale=1.0)
        z3 = prec.tile([8, BC], bf16, tag="z3", name="z3")
        ps3 = psrec.tile([8, BC], f32, tag="ps0", name="psfc")
        nc.tensor.matmul(ps3, fw3, z2, start=True, stop=True)
        nc.scalar.activation(out=z3, in_=ps3, func=AFT.Relu,
                             bias=fb3, scale=1.0)
        zo = prec.tile([1, BC], f32, tag="zo", name="zo")
        ps4 = psrec.tile([1, BC], f32, tag="ps0", name="psfc")
        nc.tensor.matmul(ps4, fw4, z3, start=True, stop=True)
        nc.scalar.activation(out=zo, in_=ps4, func=AFT.Relu,
                             bias=fb4, scale=1.0)
        nc.sync.dma_start(out=out, in_=zo)


# ---------------------------------------------------------------------------
# host side
# ---------------------------------------------------------------------------

def prep_inputs(x, a, conv1_w, conv1_b, conv2_w, conv2_b, conv3_w, conv3_b,
                W_ih0, W_hh0, b0, W_ih1, W_hh1, b1,
                fc1_w, fc1_b, fc2_w, fc2_b, fc3_w, fc3_b, fc4_w, fc4_b):
    """Returns list of per-core input dicts."""
    p = GATE_PERM
    f = np.float32
    S = ACT_SCALE if FP8 else 1.0
    E4 = mybir.dt.np(f8e4)

    def _whh(w):
        w = np.ascontiguousarray(S * w)
        return w.astype(E4) if FP8 else w.astype(BF)
    shared = {
        "w1": np.ascontiguousarray(np.asarray(conv1_w, f).transpose(1, 2, 0)).astype(BF),
        "w2": np.ascontiguousarray(np.asarray(conv2_w, f).transpose(1, 2, 0)).astype(BF),
        "w3": np.ascontiguousarray(np.asarray(conv3_w, f).transpose(1, 2, 0)).astype(BF),
        "cb1": np.asarray(conv1_b, f), "cb2": np.asarray(conv2_b, f),
        "cb3": np.asarray(conv3_b, f),
        "wih0": np.ascontiguousarray(S * np.asarray(W_ih0, f).T[:, p]).astype(BF),
        "whh0": _whh(np.asarray(W_hh0, f).T[:, p]),
        "bg0": S * np.asarray(b0, f)[p],
        "wih1": np.ascontiguousarray(S * np.asarray(W_ih1, f).T[:, p]).astype(BF),
        "whh1": _whh(np.asarray(W_hh1, f).T[:, p]),
        "bg1": S * np.asarray(b1, f)[p],
        "ident": np.eye(128, dtype=BF),
        "fw1": np.ascontiguousarray(np.asarray(fc1_w, f).T).astype(BF),
        "fw2": np.ascontiguousarray(np.asarray(fc2_w, f).T).astype(BF),
        "fw3": np.ascontiguousarray(np.asarray(fc3_w, f).T).astype(BF),
        "fw4": np.ascontiguousarray(np.asarray(fc4_w, f).T).astype(BF),
        "fb1": np.asarray(fc1_b, f), "fb2": np.asarray(fc2_b, f),
        "fb3": np.asarray(fc3_b, f), "fb4": np.asarray(fc4_b, f),
    }
    x = np.asarray(x, f)
    a = np.asarray(a)
    in_maps = []
    for c in range(N_CORES):
        xs = x[BC * c:BC * (c + 1)]             # [BC, CIN, T]
        xbp = np.zeros((CIN, BC, T + 2), BF)
        xbp[:, :, 1:T + 1] = xs.transpose(1, 0, 2).astype(BF)
        ash = a[BC * c:BC * (c + 1)].astype(np.int64)
        mkv = (np.arange(T2)[:, None] < ash[None, :]).astype(f)
        mkv = mkv / ash[None, :].astype(f)
        m = dict(shared)
        m["xb"] = xbp
        m["mk"] = np.ascontiguousarray(mkv.T)
        in_maps.append(m)
    return in_maps


_CACHED_NC = None


def kernel(**inputs):
    global _CACHED_NC
    if _CACHED_NC is None:
        _CACHED_NC = build_kernel()
    in_maps = prep_inputs(**inputs)
    res = run_bass_kernel_spmd(_CACHED_NC, in_maps,
                               core_ids=list(range(N_CORES)))
    z = np.concatenate([res.results[c]["out"] for c in range(N_CORES)])
    return z.reshape(B, 1).astype(np.float32)



# revision 7
# speedup vs baseline: 1.1767x; 1.1767x over previous
"""CNN-LSTM kernel for Trainium2, 8-core data parallel.

Model: Conv1d(1024->768->384->192, k=3, pad=1, ReLU) x3 -> MaxPool1d(2)
       -> 2-layer LSTM(H=512) -> masked mean over valid steps -> FC head.

Sharding: batch 64 split 8 ways (8 samples per core); weights replicated.

Layouts (per core, all channel-major so nothing needs an on-device transpose):
  x / conv activations: [C_part, batch=8, time(+2 pad)]
  LSTM gate psum:       [128, 16 blocks, 8]  (blocks host-permuted:
                        [i0,f0,o0, i1,f1,o1, ..., g0,g1,g2,g3])
  h state:              [128, 4 hchunk, 8] (bf16 ping-pong)
  masked mean: acc += h_t * (mask[t,b]/a[b]) each step.
"""

from contextlib import ExitStack

import numpy as np
import ml_dtypes

import concourse.bass as bass
import concourse.mybir as mybir
import concourse.tile as tile
from concourse import bacc
from concourse.bass import ds
from concourse.bass_utils import run_bass_kernel_spmd

BF = ml_dtypes.bfloat16
bf16 = mybir.dt.bfloat16
f32 = mybir.dt.float32
ET = mybir.EngineType
f8e4 = mybir.dt.float8e4
AFT = mybir.ActivationFunctionType

N_CORES = 8
B, BC = 64, 8
CIN, T, T2, H, G = 1024, 512, 256, 512, 2048
C1, C2, C3 = 768, 384, 192
UNROLL = 32
FP8 = True
ACT_SCALE = 8.0


# gate block order (chain-consumption order): [g0..g3 | i0,f0,i1,f1,i2,f2,
# i3,f3 | o0..o3].  psum group A = blocks 0:4 (tanh g), B = 4:12 (sigmoid
# i/f), C = 12:16 (sigmoid o).  The burst fills A, then B, then C so the
# tail's ACT ops start while the burst is still issuing, and sigmoid(o)
# overlaps the DVE c-update.
def gate_perm():
    g = lambda base, m: base + 128 * m + np.arange(128)
    blocks = [g(1024, m) for m in range(4)]          # g_m
    for m in range(4):
        blocks.append(g(0, m))                       # i_m
        blocks.append(g(512, m))                     # f_m
    blocks += [g(1536, m) for m in range(4)]         # o_m
    return np.concatenate(blocks)


GATE_PERM = gate_perm()

TAP_SPECS = {
    "y1": ([C1, BC, T], bf16), "y2": ([C2, BC, T], bf16),
    "y3": ([C3, BC, T], bf16), "pl": ([C3, BC, T2], bf16),
    "xp0": ([G, BC, T2], bf16), "h0": ([H, BC, T2], bf16),
    "xp1": ([G, BC, T2], bf16), "macc": ([H, BC], f32),
}


def build_kernel(taps=()):
    nc = bacc.Bacc("TRN2", target_bir_lowering=False, debug=False,
                   num_devices=N_CORES)
    din = {}

    def inp(name, shape, dt=bf16):
        din[name] = nc.dram_tensor(name, shape, dt, kind="ExternalInput").ap()

    inp("xb", [CIN, BC, T + 2])                 # padded time, cols 0,T+1 zero
    inp("w1", [CIN, 3, C1])
    inp("w2", [C1, 3, C2])
    inp("w3", [C2, 3, C3])
    inp("cb1", [C1], f32)
    inp("cb2", [C2], f32)
    inp("cb3", [C3], f32)
    whh_dt = f8e4 if FP8 else bf16
    inp("wih0", [C3, G])                        # gate cols permuted
    inp("whh0", [H, G], whh_dt)
    inp("bg0", [G], f32)
    inp("wih1", [H, G])
    inp("whh1", [H, G], whh_dt)
    inp("bg1", [G], f32)
    inp("fw1", [H, 256])
    inp("fw2", [256, 64])
    inp("fw3", [64, 8])
    inp("fw4", [8, 1])
    inp("fb1", [256], f32)
    inp("fb2", [64], f32)
    inp("fb3", [8], f32)
    inp("fb4", [1], f32)
    inp("mk", [BC, T2], f32)                    # mask[t,b]/a[b], b-major
    inp("ident", [128, 128])                    # identity for xp->psum fold

    out = nc.dram_tensor("out", [BC], f32, kind="ExternalOutput").ap()
    dtap = {}
    for tp in taps:
        shp, dt = TAP_SPECS[tp]
        dtap[tp] = nc.dram_tensor("tap_" + tp, shp, dt,
                                  kind="ExternalOutput").ap()

    with tile.TileContext(nc) as tc:
        _body(nc, tc, din, out, dtap)
    nc.compile()
    return nc


def _conv_layer(nc, psum, wsb, bsb, src, dst, tap):
    """src: list of [128, BC, T+2] padded tiles; dst: list of co-chunk tiles,
    padded [p, BC, T+2] (written at [:, :, 1:T+1]) or [p, BC, T].
    wsb: per ci-chunk [128, 3, co] tiles. bsb: [128, n_co] tile."""
    nci = len(src)
    pads = dst[0].shape[2] == T + 2
    for j, dtile in enumerate(dst):
        mj = dtile.shape[0]
        for b in range(BC):
            ps = psum.tile([128, T], f32, tag="pscv", name="pscv")
            n = 0
            for c in range(nci):
                for k in range(3):
                    nc.tensor.matmul(
                        ps[:mj, :],
                        wsb[c][:, k, 128 * j:128 * j + mj],
                        src[c][:, b, k:k + T],
                        start=(n == 0), stop=(n == 3 * nci - 1))
                    n += 1
            od = dtile[:, b, 1:T + 1] if pads else dtile[:, b, :]
            nc.scalar.activation(out=od, in_=ps[:mj, :], func=AFT.Relu,
                                 bias=bsb[:mj, j:j + 1], scale=1.0)
        if tap is not None:
            nc.sync.dma_start(
                out=tap[128 * j:128 * j + mj, :, :],
                in_=dtile[:, :, 1:T + 1] if pads else dtile[:, :, :])


def _proj(nc, psum, wsb, bias_sb, rhs_slices, xp, nk, tap):
    """xp[:, n, :, :] = sum_k wsb[k][:, nblk].T @ rhs_k + bias.
    wsb: nk tiles [<=128, G]; rhs_slices(k, s) -> [<=128, 2, T2] AP;
    xp: [128, 16, BC, T2] bf16; bias_sb [128, 16] f32."""
    for n in range(16):
        for s in range(BC // 2):
            ps = psum.tile([128, 2, T2], f32, tag="pscv", name="pscv")
            for k in range(nk):
                nc.tensor.matmul(ps[:, :, :],
                                 wsb[k][:, 128 * n:128 * n + 128],
                                 rhs_slices(k, s),
                                 start=(k == 0), stop=(k == nk - 1))
            dst = xp[:, n, 2 * s:2 * s + 2, :]
            if (n + s) % 2 == 0:
                nc.scalar.activation(out=dst, in_=ps, func=AFT.Identity,
                                     bias=bias_sb[:, n:n + 1], scale=1.0)
            else:
                nc.vector.tensor_scalar_add(out=dst, in0=ps,
                                            scalar1=bias_sb[:, n:n + 1])
        if tap is not None:
            nc.sync.dma_start(out=tap[128 * n:128 * (n + 1), :, :],
                              in_=xp[:, n, :, :])


def _lstm_fused(nc, tc, xp, whh0_sb, whh1_sb, wih1_sb, ident, bg1, msk4,
                macc, prec, pwin, psum, psrec, act_scale):
    """Both LSTM layers, L1 lagging L0 by one 32-step window.
    Gate blocks are host-permuted to [g|if|o] so the burst fills three psum
    groups in chain order: tanh(g) and sigmoid(i,f) run on ACT while the
    burst is still issuing, sigmoid(o) overlaps the DVE c-update.
    L0's h is written straight into h0win (no copy); the next step's burst
    streams h0win[:, k, :, u-1].  hlast carries h across window boundaries."""
    W = UNROLL
    hb1 = [prec.tile([128, 4, BC], bf16, tag=f"hb1_{p}", name=f"hb1_{p}") for p in (0, 1)]
    c0 = prec.tile([128, 4, BC], f32, tag="c0", name="c0")
    c1 = prec.tile([128, 4, BC], f32, tag="c1", name="c1")
    hlast = prec.tile([128, 4, BC], bf16, tag="hlast", name="hlast")
    h0win = pwin.tile([128, 4, BC, W], bf16, tag="h0win", name="h0win")
    xp1ring = pwin.tile([128, 16, BC, W], bf16, tag="xp1ring", name="xp1ring")
    for tl in (hb1[0], c0, c1, hlast):
        nc.vector.memset(tl, 0.0)
    inv = 1.0 / act_scale

    def tail(layer, ps3, c_st, hdst, t, u):
        psA, psB, psC = ps3
        tg = prec.tile([128, 4, BC], f32, tag=f"tg{layer}", name=f"tg{layer}")
        sif = prec.tile([128, 8, BC], f32, tag=f"sif{layer}", name=f"sif{layer}")
        so = prec.tile([128, 4, BC], f32, tag=f"so{layer}", name=f"so{layer}")
        nc.scalar.activation(out=tg, in_=psA, func=AFT.Tanh, scale=inv)
        nc.scalar.activation(out=sif, in_=psB, func=AFT.Sigmoid, scale=inv)
        nc.scalar.activation(out=so, in_=psC, func=AFT.Sigmoid, scale=inv)
        sifv = sif.rearrange("p (m q) b -> p m q b", q=2)
        t1 = prec.tile([128, 4, BC], f32, tag=f"t1{layer}", name=f"t1{layer}")
        t2 = prec.tile([128, 4, BC], f32, tag=f"t2{layer}", name=f"t2{layer}")
        nc.gpsimd.tensor_mul(t1, sifv[:, :, 1, :], c_st)
        nc.vector.tensor_mul(t2, sifv[:, :, 0, :], tg)
        nc.vector.tensor_add(c_st, t1, t2)
        tc_ = prec.tile([128, 4, BC], f32, tag=f"tc{layer}", name=f"tc{layer}")
        nc.scalar.activation(out=tc_, in_=c_st, func=AFT.Tanh)
        nc.vector.tensor_mul(hdst, so, tc_)
        if layer == 0:
            if u == W - 1:
                nc.gpsimd.tensor_copy(out=hlast, in_=hdst)
        else:
            tm = prec.tile([128, 4, BC], f32, tag="tm", name="tm")
            nc.gpsimd.tensor_mul(tm, hdst, msk4[:, :, :, ds(t, 1)])
            nc.gpsimd.tensor_add(macc, macc, tm)

    GRP = ((0, 4), (4, 12), (12, 16))

    def burst(ps3, id_rhs, whh_sb, hcol):
        for (j0, j1), ps in zip(GRP, ps3):
            nc.tensor.matmul(ps[:, :, :], ident, id_rhs(j0, j1),
                             start=True, stop=False, skip_group_check=True)
            for j in range(j0, j1):
                for k in range(4):
                    nc.tensor.matmul(ps[:, j - j0, :],
                                     whh_sb[k][:, 128 * j:128 * j + 128],
                                     hcol(k),
                                     start=False,
                                     stop=(j == j1 - 1 and k == 3),
                                     skip_group_check=True)

    def ps3_new(layer):
        return tuple(
            psrec.tile([128, j1 - j0, BC], f32, tag=f"ps{layer}{g}",
                       name=f"ps{layer}{g}")
            for g, (j0, j1) in enumerate(GRP))

    def step_l0(t, u):
        ps3 = ps3_new(0)
        if u == 0:
            hcol = lambda k: hlast[:, k, :]
        else:
            hcol = lambda k: h0win[:, k, :, u - 1]
        burst(ps3, lambda j0, j1: xp[:, j0:j1, :, ds(t, 1)], whh0_sb, hcol)
        tail(0, ps3, c0, h0win[:, :, :, u], t, u)

    def step_l1(t, u, par):
        ps3 = ps3_new(1)
        burst(ps3, lambda j0, j1: xp1ring[:, j0:j1, :, u], whh1_sb,
              lambda k: hb1[par][:, k, :])
        tail(1, ps3, c1, hb1[1 - par], t, u)

    def xp1batch():
        for n in range(16):
            ps = psum.tile([128, BC, W], f32, tag="pscv", name="pscv")
            for k in range(4):
                nc.tensor.matmul(ps[:, :, :],
                                 wih1_sb[k][:, 128 * n:128 * n + 128],
                                 h0win[:, k, :, :],
                                 start=(k == 0), stop=(k == 3))
            dst = xp1ring[:, n, :, :]
            if n % 2 == 0:
                nc.scalar.activation(out=dst, in_=ps, func=AFT.Identity,
                                     bias=bg1[:, n:n + 1], scale=1.0)
            else:
                nc.vector.tensor_scalar_add(out=dst, in0=ps,
                                            scalar1=bg1[:, n:n + 1])

    # prologue window: L0 steps 0..W-1, then first xp1 batch
    for u in range(W):
        step_l0(u, u)
    xp1batch()
    # main: windows with L1 lagging one window
    with tc.For_i(W, T2, W, staggered_reset=True,
                  hint_engines=(ET.PE, ET.Activation, ET.DVE, ET.Pool)) as iv:
        for u in range(W):
            step_l0(iv + u, u)
            step_l1(iv + u - W, u, u & 1)
        xp1batch()
    # epilogue: last window of L1
    for u in range(W):
        step_l1(T2 - W + u, u, u & 1)


def _body(nc, tc, din, out, dtap):
    with ExitStack() as top:
        pmisc = top.enter_context(tc.tile_pool(name="pmisc", bufs=1))
        psum = top.enter_context(tc.tile_pool(name="psum", bufs=2, space="PSUM"))
        psrec = top.enter_context(tc.tile_pool(name="psrec", bufs=1, space="PSUM"))
        prec = top.enter_context(tc.tile_pool(name="prec", bufs=2))
        ppl = top.enter_context(tc.tile_pool(name="ppl", bufs=1))

        # small persistent constants
        cb1 = pmisc.tile([128, 6], f32)
        cb2 = pmisc.tile([128, 3], f32)
        cb3 = pmisc.tile([128, 2], f32)
        nc.sync.dma_start(out=cb1, in_=din["cb1"].rearrange("(j p) -> p j", p=128))
        nc.sync.dma_start(out=cb2, in_=din["cb2"].rearrange("(j p) -> p j", p=128))
        nc.vector.memset(cb3, 0.0)
        nc.sync.dma_start(out=cb3[:, 0:1], in_=din["cb3"][0:128])
        nc.sync.dma_start(out=cb3[0:64, 1:2], in_=din["cb3"][128:192])
        bg0 = pmisc.tile([128, 16], f32)
        bg1 = pmisc.tile([128, 16], f32)
        nc.sync.dma_start(out=bg0, in_=din["bg0"].rearrange("(j p) -> p j", p=128))
        nc.sync.dma_start(out=bg1, in_=din["bg1"].rearrange("(j p) -> p j", p=128))

        # pool stack: ppl(top-level) > py2 > py1 > {px,pw1}, {pw2}, ...
        st2 = ExitStack()   # py2: closes after conv3
        st1 = ExitStack()   # py1: closes after conv2
        st0 = ExitStack()   # px + pw1: closes after conv1
        py2 = st2.enter_context(tc.tile_pool(name="py2", bufs=1))
        py1 = st1.enter_context(tc.tile_pool(name="py1", bufs=1))
        px = st0.enter_context(tc.tile_pool(name="px", bufs=1))
        pw1 = st0.enter_context(tc.tile_pool(name="pw1", bufs=1))

        # ---------------- conv1 ----------------
        y1 = [py1.tile([128, BC, T + 2], bf16, tag=f"y1_{j}", name=f"y1_{j}")
              for j in range(6)]
        for yt in y1:
            nc.gpsimd.memset(yt[:, :, 0:1], 0.0)
            nc.gpsimd.memset(yt[:, :, T + 1:T + 2], 0.0)
        xsb = [px.tile([128, BC, T + 2], bf16, tag=f"x_{c}", name=f"x_{c}")
               for c in range(8)]
        w1sb = [pw1.tile([128, 3, C1], bf16, tag=f"w1_{c}", name=f"w1_{c}")
                for c in range(8)]
        for c in range(8):
            nc.sync.dma_start(out=xsb[c], in_=din["xb"][128 * c:128 * (c + 1)])
            nc.sync.dma_start(out=w1sb[c], in_=din["w1"][128 * c:128 * (c + 1)])
        _conv_layer(nc, psum, w1sb, cb1, xsb, y1, dtap.get("y1"))
        st0.close()

        # ---------------- conv2 ----------------
        stw2 = ExitStack()
        pw2 = stw2.enter_context(tc.tile_pool(name="pw2", bufs=1))
        y2 = [py2.tile([128, BC, T + 2], bf16, tag=f"y2_{j}", name=f"y2_{j}")
              for j in range(3)]
        for yt in y2:
            nc.gpsimd.memset(yt[:, :, 0:1], 0.0)
            nc.gpsimd.memset(yt[:, :, T + 1:T + 2], 0.0)
        w2sb = [pw2.tile([128, 3, C2], bf16, tag=f"w2_{c}", name=f"w2_{c}")
                for c in range(6)]
        for c in range(6):
            nc.sync.dma_start(out=w2sb[c], in_=din["w2"][128 * c:128 * (c + 1)])
        _conv_layer(nc, psum, w2sb, cb2, y1, y2, dtap.get("y2"))
        stw2.close()
        st1.close()

        # ---------------- conv3 + maxpool ----------------
        st3 = ExitStack()
        py3 = st3.enter_context(tc.tile_pool(name="py3", bufs=1))
        pw3 = st3.enter_context(tc.tile_pool(name="pw3", bufs=1))
        y3 = [py3.tile([128, BC, T], bf16, tag="y3_0", name="y3_0"),
              py3.tile([64, BC, T], bf16, tag="y3_1", name="y3_1")]
        w3sb = [pw3.tile([128, 3, C3], bf16, tag=f"w3_{c}", name=f"w3_{c}")
                for c in range(3)]
        for c in range(3):
            nc.sync.dma_start(out=w3sb[c], in_=din["w3"][128 * c:128 * (c + 1)])
        _conv_layer(nc, psum, w3sb, cb3, y2, y3, dtap.get("y3"))

        pl = [ppl.tile([128, BC, T2], bf16, tag="pl_0", name="pl_0"),
              ppl.tile([64, BC, T2], bf16, tag="pl_1", name="pl_1")]
        for j in range(2):
            yr = y3[j].rearrange("p b (t k) -> p b t k", k=2)
            nc.vector.tensor_max(pl[j], yr[:, :, :, 0], yr[:, :, :, 1])
            if "pl" in dtap:
                nc.sync.dma_start(
                    out=dtap["pl"][128 * j:128 * j + pl[j].shape[0]], in_=pl[j])
        st3.close()
        st2.close()

        # ---------------- LSTM weights + xp0 ----------------
        plstm = top.enter_context(tc.tile_pool(name="plstm", bufs=1))
        pxp = top.enter_context(tc.tile_pool(name="pxp", bufs=1))

        wih0 = [plstm.tile([128, G], bf16, tag="wih0_0", name="wih0_0"),
                plstm.tile([64, G], bf16, tag="wih0_1", name="wih0_1")]
        nc.sync.dma_start(out=wih0[0], in_=din["wih0"][0:128])
        nc.sync.dma_start(out=wih0[1], in_=din["wih0"][128:192])
        whh_dt = f8e4 if FP8 else bf16
        whh0 = [plstm.tile([128, G], whh_dt, tag=f"whh0_{k}", name=f"whh0_{k}")
                for k in range(4)]
        wih1 = [plstm.tile([128, G], bf16, tag=f"wih1_{k}", name=f"wih1_{k}")
                for k in range(4)]
        whh1 = [plstm.tile([128, G], whh_dt, tag=f"whh1_{k}", name=f"whh1_{k}")
                for k in range(4)]
        for k in range(4):
            nc.sync.dma_start(out=whh0[k], in_=din["whh0"][128 * k:128 * (k + 1)])
            nc.sync.dma_start(out=wih1[k], in_=din["wih1"][128 * k:128 * (k + 1)])
            nc.sync.dma_start(out=whh1[k], in_=din["whh1"][128 * k:128 * (k + 1)])

        ident = plstm.tile([128, 128], bf16, tag="ident", name="ident")
        nc.sync.dma_start(out=ident, in_=din["ident"])
        msk4 = pxp.tile([128, 4, BC, T2], f32, tag="msk4", name="msk4")
        mk_ap = din["mk"]
        nc.sync.dma_start(out=msk4[:, 0, :, :], in_=bass.AP(
            tensor=mk_ap.tensor, offset=mk_ap.offset,
            ap=[[0, 128]] + list(mk_ap.ap)))
        nc.vector.tensor_copy(out=msk4[:, 1, :, :], in_=msk4[:, 0, :, :])
        nc.vector.tensor_copy(out=msk4[:, 2:4, :, :], in_=msk4[:, 0:2, :, :])
        xp = pxp.tile([128, 16, BC, T2], bf16, tag="xp", name="xp")

        _proj(nc, psum, wih0, bg0,
              lambda k, s: pl[k][:, 2 * s:2 * s + 2, :], xp, 2,
              dtap.get("xp0"))

        # ---------------- fused 2-layer LSTM ----------------
        macc = pmisc.tile([128, 4, BC], f32)
        nc.vector.memset(macc, 0.0)
        _lstm_fused(nc, tc, xp, whh0, whh1, wih1, ident, bg1, msk4, macc,
                    prec, pxp, psum, psrec, ACT_SCALE if FP8 else 1.0)
        if "macc" in dtap:
            for m in range(4):
                nc.sync.dma_start(out=dtap["macc"][128 * m:128 * (m + 1)],
                                  in_=macc[:, m, :])

        # ---------------- FC head ----------------
        fw1 = plstm.tile([128, 4, 256], bf16, tag="fw1", name="fw1")
        fw2 = plstm.tile([128, 2, 64], bf16, tag="fw2", name="fw2")
        fw3 = plstm.tile([64, 8], bf16, tag="fw3", name="fw3")
        fw4 = plstm.tile([8, 1], bf16, tag="fw4", name="fw4")
        nc.sync.dma_start(out=fw1,
                          in_=din["fw1"].rearrange("(k p) m -> p k m", p=128))
        nc.sync.dma_start(out=fw2,
                          in_=din["fw2"].rearrange("(k p) m -> p k m", p=128))
        nc.sync.dma_start(out=fw3, in_=din["fw3"])
        nc.sync.dma_start(out=fw4, in_=din["fw4"])
        fb1 = pmisc.tile([128, 2], f32)
        fb2 = pmisc.tile([64, 1], f32)
        fb3 = pmisc.tile([8, 1], f32)
        fb4 = pmisc.tile([1, 1], f32)
        nc.sync.dma_start(out=fb1,
                          in_=din["fb1"].rearrange("(j p) -> p j", p=128))
        nc.sync.dma_start(out=fb2, in_=din["fb2"])
        nc.sync.dma_start(out=fb3, in_=din["fb3"])
        nc.sync.dma_start(out=fb4, in_=din["fb4"])

        maccb = prec.tile([128, 4, BC], bf16, tag="maccb", name="maccb")
        nc.vector.tensor_copy(out=maccb, in_=macc)
        z1 = prec.tile([128, 2, BC], bf16, tag="z1", name="z1")
        for mj in range(2):
            ps = psrec.tile([128, BC], f32, tag="ps00", name="psfc")
            for k in range(4):
                nc.tensor.matmul(ps, fw1[:, k, 128 * mj:128 * mj + 128],
                                 maccb[:, k, :], start=(k == 0),
                                 stop=(k == 3))
            nc.scalar.activation(out=z1[:, mj, :], in_=ps, func=AFT.Relu,
                                 bias=fb1[:, mj:mj + 1], scale=1.0)
        z2 = prec.tile([64, BC], bf16, tag="z2", name="z2")
        ps2 = psrec.tile([64, BC], f32, tag="ps00", name="psfc")
        for k in range(2):
            nc.tensor.matmul(ps2, fw2[:, k, 0:64], z1[:, k, :],
                             start=(k == 0), stop=(k == 1))
        nc.scalar.activation(out=z2, in_=ps2, func=AFT.Relu,
                             bias=fb2, scale=1.0)
        z3 = prec.tile([8, BC], bf16, tag="z3", name="z3")
        ps3 = psrec.tile([8, BC], f32, tag="ps00", name="psfc")
        nc.tensor.matmul(ps3, fw3, z2, start=True, stop=True)
        nc.scalar.activation(out=z3, in_=ps3, func=AFT.Relu,
                             bias=fb3, scale=1.0)
        zo = prec.tile([1, BC], f32, tag="zo", name="zo")
        ps4 = psrec.tile([1, BC], f32, tag="ps00", name="psfc")
        nc.tensor.matmul(ps4, fw4, z3, start=True, stop=True)
        nc.scalar.activation(out=zo, in_=ps4, func=AFT.Relu,
                             bias=fb4, scale=1.0)
        nc.sync.dma_start(out=out, in_=zo)


# ---------------------------------------------------------------------------
# host side
# ---------------------------------------------------------------------------

def prep_inputs(x, a, conv1_w, conv1_b, conv2_w, conv2_b, conv3_w, conv3_b,
                W_ih0, W_hh0, b0, W_ih1, W_hh1, b1,
                fc1_w, fc1_b, fc2_w, fc2_b, fc3_w, fc3_b, fc4_w, fc4_b):
    """Returns list of per-core input dicts."""
    p = GATE_PERM
    f = np.float32
    S = ACT_SCALE if FP8 else 1.0
    E4 = mybir.dt.np(f8e4)

    def _whh(w):
        w = np.ascontiguousarray(S * w)
        return w.astype(E4) if FP8 else w.astype(BF)
    shared = {
        "w1": np.ascontiguousarray(np.asarray(conv1_w, f).transpose(1, 2, 0)).astype(BF),
        "w2": np.ascontiguousarray(np.asarray(conv2_w, f).transpose(1, 2, 0)).astype(BF),
        "w3": np.ascontiguousarray(np.asarray(conv3_w, f).transpose(1, 2, 0)).astype(BF),
        "cb1": np.asarray(conv1_b, f), "cb2": np.asarray(conv2_b, f),
        "cb3": np.asarray(conv3_b, f),
        "wih0": np.ascontiguousarray(S * np.asarray(W_ih0, f).T[:, p]).astype(BF),
        "whh0": _whh(np.asarray(W_hh0, f).T[:, p]),
        "bg0": S * np.asarray(b0, f)[p],
        "wih1": np.ascontiguousarray(S * np.asarray(W_ih1, f).T[:, p]).astype(BF),
        "whh1": _whh(np.asarray(W_hh1, f).T[:, p]),
        "bg1": S * np.asarray(b1, f)[p],
        "ident": np.eye(128, dtype=BF),
        "fw1": np.ascontiguousarray(np.asarray(fc1_w, f).T).astype(BF),
        "fw2": np.ascontiguousarray(np.asarray(fc2_w, f).T).astype(BF),
        "fw3": np.ascontiguousarray(np.asarray(fc3_w, f).T).astype(BF),
        "fw4": np.ascontiguousarray(np.asarray(fc4_w, f).T).astype(BF),
        "fb1": np.asarray(fc1_b, f), "fb2": np.asarray(fc2_b, f),
        "fb3": np.asarray(fc3_b, f), "fb4": np.asarray(fc4_b, f),
    }
    x = np.asarray(x, f)
    a = np.asarray(a)
    in_maps = []
    for c in range(N_CORES):
        xs = x[BC * c:BC * (c + 1)]             # [BC, CIN, T]
        xbp = np.zeros((CIN, BC, T + 2), BF)
        xbp[:, :, 1:T + 1] = xs.transpose(1, 0, 2).astype(BF)
        ash = a[BC * c:BC * (c + 1)].astype(np.int64)
        mkv = (np.arange(T2)[:, None] < ash[None, :]).astype(f)
        mkv = mkv / ash[None, :].astype(f)
        m = dict(shared)
        m["xb"] = xbp
        m["mk"] = np.ascontiguousarray(mkv.T)
        in_maps.append(m)
    return in_maps


_CACHED_NC = None


def kernel(**inputs):
    global _CACHED_NC
    if _CACHED_NC is None:
        _CACHED_NC = build_kernel()
    in_maps = prep_inputs(**inputs)
    res = run_bass_kernel_spmd(_CACHED_NC, in_maps,
                               core_ids=list(range(N_CORES)))
    z = np.concatenate([res.results[c]["out"] for c in range(N_CORES)])
    return z.reshape(B, 1).astype(np.float32)



# revision 10
# speedup vs baseline: 1.1838x; 1.0061x over previous
"""CNN-LSTM kernel for Trainium2, 8-core data parallel.

Model: Conv1d(1024->768->384->192, k=3, pad=1, ReLU) x3 -> MaxPool1d(2)
       -> 2-layer LSTM(H=512) -> masked mean over valid steps -> FC head.

Sharding: batch 64 split 8 ways (8 samples per core); weights replicated.

Layouts (per core, all channel-major so nothing needs an on-device transpose):
  x / conv activations: [C_part, batch=8, time(+2 pad)]
  LSTM gate psum:       [128, 16 blocks, 8]  (blocks host-permuted:
                        [i0,f0,o0, i1,f1,o1, ..., g0,g1,g2,g3])
  h state:              [128, 4 hchunk, 8] (bf16 ping-pong)
  masked mean: acc += h_t * (mask[t,b]/a[b]) each step.
"""

from contextlib import ExitStack

import numpy as np
import ml_dtypes

import concourse.bass as bass
import concourse.mybir as mybir
import concourse.tile as tile
from concourse import bacc
from concourse.bass import ds
from concourse.bass_utils import run_bass_kernel_spmd

BF = ml_dtypes.bfloat16
bf16 = mybir.dt.bfloat16
f32 = mybir.dt.float32
ET = mybir.EngineType
f8e4 = mybir.dt.float8e4
AFT = mybir.ActivationFunctionType

N_CORES = 8
B, BC = 64, 8
CIN, T, T2, H, G = 1024, 512, 256, 512, 2048
C1, C2, C3 = 768, 384, 192
UNROLL = 32
FP8 = True
ACT_SCALE = 8.0


# gate block order (chain-consumption order): [g0..g3 | i0,f0,i1,f1,i2,f2,
# i3,f3 | o0..o3].  psum group A = blocks 0:4 (tanh g), B = 4:12 (sigmoid
# i/f), C = 12:16 (sigmoid o).  The burst fills A, then B, then C so the
# tail's ACT ops start while the burst is still issuing, and sigmoid(o)
# overlaps the DVE c-update.
def gate_perm():
    g = lambda base, m: base + 128 * m + np.arange(128)
    blocks = [g(1024, m) for m in range(4)]          # g_m
    for m in range(4):
        blocks.append(g(0, m))                       # i_m
        blocks.append(g(512, m))                     # f_m
    blocks += [g(1536, m) for m in range(4)]         # o_m
    return np.concatenate(blocks)


GATE_PERM = gate_perm()

TAP_SPECS = {
    "y1": ([C1, BC, T], bf16), "y2": ([C2, BC, T], bf16),
    "y3": ([C3, BC, T], bf16), "pl": ([C3, BC, T2], bf16),
    "xp0": ([G, BC, T2], bf16), "h0": ([H, BC, T2], bf16),
    "xp1": ([G, BC, T2], bf16), "macc": ([H, BC], f32),
}


def build_kernel(taps=()):
    nc = bacc.Bacc("TRN2", target_bir_lowering=False, debug=False,
                   num_devices=N_CORES)
    din = {}

    def inp(name, shape, dt=bf16):
        din[name] = nc.dram_tensor(name, shape, dt, kind="ExternalInput").ap()

    inp("xb", [CIN, BC, T + 2])                 # padded time, cols 0,T+1 zero
    inp("w1", [CIN, 3, C1])
    inp("w2", [C1, 3, C2])
    inp("w3", [C2, 3, C3])
    inp("cb1", [C1], f32)
    inp("cb2", [C2], f32)
    inp("cb3", [C3], f32)
    whh_dt = f8e4 if FP8 else bf16
    inp("wih0", [C3, G])                        # gate cols permuted
    inp("whh0", [H, G], whh_dt)
    inp("bg0", [G], f32)
    inp("wih1", [H, G])
    inp("whh1", [H, G], whh_dt)
    inp("bg1", [G], f32)
    inp("fw1", [H, 256])
    inp("fw2", [256, 64])
    inp("fw3", [64, 8])
    inp("fw4", [8, 1])
    inp("fb1", [256], f32)
    inp("fb2", [64], f32)
    inp("fb3", [8], f32)
    inp("fb4", [1], f32)
    inp("mk", [BC, T2], f32)                    # mask[t,b]/a[b], b-major
    inp("ident", [128, 128])                    # identity for xp->psum fold

    out = nc.dram_tensor("out", [BC], f32, kind="ExternalOutput").ap()
    dtap = {}
    for tp in taps:
        shp, dt = TAP_SPECS[tp]
        dtap[tp] = nc.dram_tensor("tap_" + tp, shp, dt,
                                  kind="ExternalOutput").ap()

    with tile.TileContext(nc) as tc:
        _body(nc, tc, din, out, dtap)
    nc.compile()
    return nc


def _conv_layer(nc, psum, wsb, bsb, src, dst, tap):
    """src: list of [128, BC, T+2] padded tiles; dst: list of co-chunk tiles,
    padded [p, BC, T+2] (written at [:, :, 1:T+1]) or [p, BC, T].
    wsb: per ci-chunk [128, 3, co] tiles. bsb: [128, n_co] tile."""
    nci = len(src)
    pads = dst[0].shape[2] == T + 2
    for j, dtile in enumerate(dst):
        mj = dtile.shape[0]
        for b in range(BC):
            ps = psum.tile([128, T], f32, tag="pscv", name="pscv")
            n = 0
            for c in range(nci):
                for k in range(3):
                    nc.tensor.matmul(
                        ps[:mj, :],
                        wsb[c][:, k, 128 * j:128 * j + mj],
                        src[c][:, b, k:k + T],
                        start=(n == 0), stop=(n == 3 * nci - 1))
                    n += 1
            od = dtile[:, b, 1:T + 1] if pads else dtile[:, b, :]
            nc.scalar.activation(out=od, in_=ps[:mj, :], func=AFT.Relu,
                                 bias=bsb[:mj, j:j + 1], scale=1.0)
        if tap is not None:
            nc.sync.dma_start(
                out=tap[128 * j:128 * j + mj, :, :],
                in_=dtile[:, :, 1:T + 1] if pads else dtile[:, :, :])


def _proj(nc, psum, wsb, bias_sb, rhs_slices, xp, nk, tap):
    """xp[:, n, :, :] = sum_k wsb[k][:, nblk].T @ rhs_k + bias.
    wsb: nk tiles [<=128, G]; rhs_slices(k, s) -> [<=128, 2, T2] AP;
    xp: [128, 16, BC, T2] bf16; bias_sb [128, 16] f32."""
    for n in range(16):
        for s in range(BC // 2):
            ps = psum.tile([128, 2, T2], f32, tag="pscv", name="pscv")
            for k in range(nk):
                nc.tensor.matmul(ps[:, :, :],
                                 wsb[k][:, 128 * n:128 * n + 128],
                                 rhs_slices(k, s),
                                 start=(k == 0), stop=(k == nk - 1))
            dst = xp[:, n, 2 * s:2 * s + 2, :]
            if (n + s) % 2 == 0:
                nc.scalar.activation(out=dst, in_=ps, func=AFT.Identity,
                                     bias=bias_sb[:, n:n + 1], scale=1.0)
            else:
                nc.vector.tensor_scalar_add(out=dst, in0=ps,
                                            scalar1=bias_sb[:, n:n + 1])
        if tap is not None:
            nc.sync.dma_start(out=tap[128 * n:128 * (n + 1), :, :],
                              in_=xp[:, n, :, :])


LAG = 24          # L1 trails L0 by this many steps
XP0_US = {5: 0, 9: 4, 13: 8, 21: 12}   # u -> first n-block of xp0 part


def _lstm_fused(nc, tc, xp, pl, wih0_sb, whh0_sb, whh1_sb, wih1_sb, ident,
                bg0, bg1, msk4, macc, prec, pwin, psum, psrec, act_scale):
    """Both LSTM layers, L1 lagging L0 by LAG steps.
    Gate blocks are host-permuted to [g|if|o] so the burst fills three psum
    groups in chain order: tanh(g) and sigmoid(i,f) run on ACT while the
    burst is still issuing, sigmoid(o) overlaps the DVE c-update.
    L0's h is written straight into the h0win ring (no copy); the next
    step's burst streams h0win[:, k, :, (u-1)%W].  xp1 is computed in
    16-col chunks and xp0 in 32-col chunks interleaved into the step
    stream, so the PE fills its dependency gaps with projection work."""
    W = UNROLL
    hb1 = [prec.tile([128, 4, BC], bf16, tag=f"hb1_{p}", name=f"hb1_{p}") for p in (0, 1)]
    c0 = prec.tile([128, 4, BC], f32, tag="c0", name="c0")
    c1 = prec.tile([128, 4, BC], f32, tag="c1", name="c1")
    h0win = pwin.tile([128, 4, BC, W], bf16, tag="h0win", name="h0win")
    xp1ring = pwin.tile([128, 16, BC, W], bf16, tag="xp1ring", name="xp1ring")
    for tl in (hb1[0], c0, c1, h0win):
        nc.vector.memset(tl, 0.0)
    inv = 1.0 / act_scale

    def tail(layer, ps3, c_st, hdst, t):
        psA, psB, psC = ps3
        tg = prec.tile([128, 4, BC], f32, tag=f"tg{layer}", name=f"tg{layer}")
        sif = prec.tile([128, 8, BC], f32, tag=f"sif{layer}", name=f"sif{layer}")
        so = prec.tile([128, 4, BC], f32, tag=f"so{layer}", name=f"so{layer}")
        nc.scalar.activation(out=tg, in_=psA, func=AFT.Tanh, scale=inv)
        nc.scalar.activation(out=sif, in_=psB, func=AFT.Sigmoid, scale=inv)
        nc.scalar.activation(out=so, in_=psC, func=AFT.Sigmoid, scale=inv)
        sifv = sif.rearrange("p (m q) b -> p m q b", q=2)
        t1 = prec.tile([128, 4, BC], f32, tag=f"t1{layer}", name=f"t1{layer}")
        t2 = prec.tile([128, 4, BC], f32, tag=f"t2{layer}", name=f"t2{layer}")
        nc.gpsimd.tensor_mul(t1, sifv[:, :, 1, :], c_st)
        nc.vector.tensor_mul(t2, sifv[:, :, 0, :], tg)
        nc.vector.tensor_add(c_st, t1, t2)
        tc_ = prec.tile([128, 4, BC], f32, tag=f"tc{layer}", name=f"tc{layer}")
        nc.scalar.activation(out=tc_, in_=c_st, func=AFT.Tanh)
        nc.vector.tensor_mul(hdst, so, tc_)
        if layer == 1:
            tm = prec.tile([128, 4, BC], f32, tag="tm", name="tm")
            nc.gpsimd.tensor_mul(tm, hdst, msk4[:, :, :, ds(t, 1)])
            nc.gpsimd.tensor_add(macc, macc, tm)

    GRP = ((0, 4), (4, 12), (12, 16))

    def burst(ps3, id_rhs, whh_sb, hcol):
        for (j0, j1), ps in zip(GRP, ps3):
            nc.tensor.matmul(ps[:, :, :], ident, id_rhs(j0, j1),
                             start=True, stop=False, skip_group_check=True)
            for j in range(j0, j1):
                for k in range(4):
                    nc.tensor.matmul(ps[:, j - j0, :],
                                     whh_sb[k][:, 128 * j:128 * j + 128],
                                     hcol(k),
                                     start=False,
                                     stop=(j == j1 - 1 and k == 3),
                                     skip_group_check=True)

    def ps3_new(layer):
        return tuple(
            psrec.tile([128, j1 - j0, BC], f32, tag=f"ps{layer}{g}",
                       name=f"ps{layer}{g}")
            for g, (j0, j1) in enumerate(GRP))

    def step_l0(t, u):
        ps3 = ps3_new(0)
        hcol = lambda k: h0win[:, k, :, (u + W - 1) % W]
        burst(ps3, lambda j0, j1: xp[:, j0:j1, :, ds(t, 1)], whh0_sb, hcol)
        tail(0, ps3, c0, h0win[:, :, :, u], t)

    def step_l1(t, col, par):
        ps3 = ps3_new(1)
        burst(ps3, lambda j0, j1: xp1ring[:, j0:j1, :, col], whh1_sb,
              lambda k: hb1[par][:, k, :])
        tail(1, ps3, c1, hb1[1 - par], t)

    def xp1chunk(c0_, half):
        for n in range(8 * half, 8 * half + 8):
            psq = psum.tile([128, BC, 16], f32, tag="pscv", name="pscv")
            for k in range(4):
                nc.tensor.matmul(psq[:, :, :],
                                 wih1_sb[k][:, 128 * n:128 * n + 128],
                                 h0win[:, k, :, c0_:c0_ + 16],
                                 start=(k == 0), stop=(k == 3))
            nc.vector.tensor_scalar_add(out=xp1ring[:, n, :, c0_:c0_ + 16],
                                        in0=psq, scalar1=bg1[:, n:n + 1])

    def xp0part(tsl, n0):
        for n in range(n0, n0 + 4):
            ps = psum.tile([128, BC, W], f32, tag="pscv", name="pscv")
            for k in range(2):
                nc.tensor.matmul(ps[:, :, :],
                                 wih0_sb[k][:, 128 * n:128 * n + 128],
                                 pl[k][:, :, tsl],
                                 start=(k == 0), stop=(k == 1))
            nc.vector.tensor_scalar_add(out=xp[:, n, :, tsl], in0=ps,
                                        scalar1=bg0[:, n:n + 1])

    def body(t0, first):
        for u in range(W):
            step_l0(t0 + u, u)
            if (not first) or u >= LAG:
                step_l1(t0 + u - LAG, (u + W - LAG) % W, u & 1)
            if u in (2, 3) and not first:
                xp1chunk(16, u - 2)
            if u in (18, 19):
                xp1chunk(0, u - 18)
            if u in XP0_US:
                xp0part(ds(t0 + W, W), XP0_US[u])

    # xp0 for window 0, then prologue body (no L1 for u < LAG)
    for n0 in (0, 4, 8, 12):
        xp0part(ds(0, W), n0)
    body(0, True)
    with tc.For_i(W, T2, W, staggered_reset=True,
                  hint_engines=(ET.PE, ET.Activation, ET.DVE, ET.Pool)) as iv:
        body(iv, False)
    # epilogue: last LAG steps of L1
    for t in range(T2 - LAG, T2 - 16):
        step_l1(t, t % W, t & 1)
    xp1chunk(16, 0)
    xp1chunk(16, 1)
    for t in range(T2 - 16, T2):
        step_l1(t, t % W, t & 1)


def _body(nc, tc, din, out, dtap):
    with ExitStack() as top:
        pmisc = top.enter_context(tc.tile_pool(name="pmisc", bufs=1))
        psum = top.enter_context(tc.tile_pool(name="psum", bufs=2, space="PSUM"))
        psrec = top.enter_context(tc.tile_pool(name="psrec", bufs=1, space="PSUM"))
        prec = top.enter_context(tc.tile_pool(name="prec", bufs=2))
        ppl = top.enter_context(tc.tile_pool(name="ppl", bufs=1))

        # small persistent constants
        cb1 = pmisc.tile([128, 6], f32)
        cb2 = pmisc.tile([128, 3], f32)
        cb3 = pmisc.tile([128, 2], f32)
        nc.sync.dma_start(out=cb1, in_=din["cb1"].rearrange("(j p) -> p j", p=128))
        nc.sync.dma_start(out=cb2, in_=din["cb2"].rearrange("(j p) -> p j", p=128))
        nc.vector.memset(cb3, 0.0)
        nc.sync.dma_start(out=cb3[:, 0:1], in_=din["cb3"][0:128])
        nc.sync.dma_start(out=cb3[0:64, 1:2], in_=din["cb3"][128:192])
        bg0 = pmisc.tile([128, 16], f32)
        bg1 = pmisc.tile([128, 16], f32)
        nc.sync.dma_start(out=bg0, in_=din["bg0"].rearrange("(j p) -> p j", p=128))
        nc.sync.dma_start(out=bg1, in_=din["bg1"].rearrange("(j p) -> p j", p=128))

        # pool stack: ppl(top-level) > py2 > py1 > {px,pw1}, {pw2}, ...
        st2 = ExitStack()   # py2: closes after conv3
        st1 = ExitStack()   # py1: closes after conv2
        st0 = ExitStack()   # px + pw1: closes after conv1
        py2 = st2.enter_context(tc.tile_pool(name="py2", bufs=1))
        py1 = st1.enter_context(tc.tile_pool(name="py1", bufs=1))
        px = st0.enter_context(tc.tile_pool(name="px", bufs=1))
        pw1 = st0.enter_context(tc.tile_pool(name="pw1", bufs=1))

        # ---------------- conv1 ----------------
        y1 = [py1.tile([128, BC, T + 2], bf16, tag=f"y1_{j}", name=f"y1_{j}")
              for j in range(6)]
        for yt in y1:
            nc.gpsimd.memset(yt[:, :, 0:1], 0.0)
            nc.gpsimd.memset(yt[:, :, T + 1:T + 2], 0.0)
        xsb = [px.tile([128, BC, T + 2], bf16, tag=f"x_{c}", name=f"x_{c}")
               for c in range(8)]
        w1sb = [pw1.tile([128, 3, C1], bf16, tag=f"w1_{c}", name=f"w1_{c}")
                for c in range(8)]
        for c in range(8):
            nc.sync.dma_start(out=xsb[c], in_=din["xb"][128 * c:128 * (c + 1)])
            nc.sync.dma_start(out=w1sb[c], in_=din["w1"][128 * c:128 * (c + 1)])
        _conv_layer(nc, psum, w1sb, cb1, xsb, y1, dtap.get("y1"))
        st0.close()

        # ---------------- conv2 ----------------
        stw2 = ExitStack()
        pw2 = stw2.enter_context(tc.tile_pool(name="pw2", bufs=1))
        y2 = [py2.tile([128, BC, T + 2], bf16, tag=f"y2_{j}", name=f"y2_{j}")
              for j in range(3)]
        for yt in y2:
            nc.gpsimd.memset(yt[:, :, 0:1], 0.0)
            nc.gpsimd.memset(yt[:, :, T + 1:T + 2], 0.0)
        w2sb = [pw2.tile([128, 3, C2], bf16, tag=f"w2_{c}", name=f"w2_{c}")
                for c in range(6)]
        for c in range(6):
            nc.sync.dma_start(out=w2sb[c], in_=din["w2"][128 * c:128 * (c + 1)])
        _conv_layer(nc, psum, w2sb, cb2, y1, y2, dtap.get("y2"))
        stw2.close()
        st1.close()

        # ---------------- conv3 + maxpool ----------------
        st3 = ExitStack()
        py3 = st3.enter_context(tc.tile_pool(name="py3", bufs=1))
        pw3 = st3.enter_context(tc.tile_pool(name="pw3", bufs=1))
        y3 = [py3.tile([128, BC, T], bf16, tag="y3_0", name="y3_0"),
              py3.tile([64, BC, T], bf16, tag="y3_1", name="y3_1")]
        w3sb = [pw3.tile([128, 3, C3], bf16, tag=f"w3_{c}", name=f"w3_{c}")
                for c in range(3)]
        for c in range(3):
            nc.sync.dma_start(out=w3sb[c], in_=din["w3"][128 * c:128 * (c + 1)])
        _conv_layer(nc, psum, w3sb, cb3, y2, y3, dtap.get("y3"))

        pl = [ppl.tile([128, BC, T2 + 32], bf16, tag="pl_0", name="pl_0"),
              ppl.tile([64, BC, T2 + 32], bf16, tag="pl_1", name="pl_1")]
        for j in range(2):
            nc.gpsimd.memset(pl[j][:, :, T2:], 0.0)
            yr = y3[j].rearrange("p b (t k) -> p b t k", k=2)
            nc.vector.tensor_max(pl[j][:, :, 0:T2], yr[:, :, :, 0],
                                 yr[:, :, :, 1])
            if "pl" in dtap:
                nc.sync.dma_start(
                    out=dtap["pl"][128 * j:128 * j + pl[j].shape[0]],
                    in_=pl[j][:, :, 0:T2])
        st3.close()
        st2.close()

        # ---------------- LSTM weights + xp0 ----------------
        plstm = top.enter_context(tc.tile_pool(name="plstm", bufs=1))
        pxp = top.enter_context(tc.tile_pool(name="pxp", bufs=1))

        wih0 = [plstm.tile([128, G], bf16, tag="wih0_0", name="wih0_0"),
                plstm.tile([64, G], bf16, tag="wih0_1", name="wih0_1")]
        nc.sync.dma_start(out=wih0[0], in_=din["wih0"][0:128])
        nc.sync.dma_start(out=wih0[1], in_=din["wih0"][128:192])
        whh_dt = f8e4 if FP8 else bf16
        whh0 = [plstm.tile([128, G], whh_dt, tag=f"whh0_{k}", name=f"whh0_{k}")
                for k in range(4)]
        wih1 = [plstm.tile([128, G], bf16, tag=f"wih1_{k}", name=f"wih1_{k}")
                for k in range(4)]
        whh1 = [plstm.tile([128, G], whh_dt, tag=f"whh1_{k}", name=f"whh1_{k}")
                for k in range(4)]
        for k in range(4):
            nc.sync.dma_start(out=whh0[k], in_=din["whh0"][128 * k:128 * (k + 1)])
            nc.sync.dma_start(out=wih1[k], in_=din["wih1"][128 * k:128 * (k + 1)])
            nc.sync.dma_start(out=whh1[k], in_=din["whh1"][128 * k:128 * (k + 1)])

        ident = plstm.tile([128, 128], bf16, tag="ident", name="ident")
        nc.sync.dma_start(out=ident, in_=din["ident"])
        msk4 = pxp.tile([128, 4, BC, T2], f32, tag="msk4", name="msk4")
        mk_ap = din["mk"]
        nc.sync.dma_start(out=msk4[:, 0, :, :], in_=bass.AP(
            tensor=mk_ap.tensor, offset=mk_ap.offset,
            ap=[[0, 128]] + list(mk_ap.ap)))
        nc.vector.tensor_copy(out=msk4[:, 1, :, :], in_=msk4[:, 0, :, :])
        nc.vector.tensor_copy(out=msk4[:, 2:4, :, :], in_=msk4[:, 0:2, :, :])
        xp = pxp.tile([128, 16, BC, T2 + 32], bf16, tag="xp", name="xp")

        # ---------------- fused 2-layer LSTM ----------------
        macc = pmisc.tile([128, 4, BC], f32)
        nc.vector.memset(macc, 0.0)
        _lstm_fused(nc, tc, xp, pl, wih0, whh0, whh1, wih1, ident, bg0, bg1,
                    msk4, macc, prec, pxp, psum, psrec,
                    ACT_SCALE if FP8 else 1.0)
        if "macc" in dtap:
            for m in range(4):
                nc.sync.dma_start(out=dtap["macc"][128 * m:128 * (m + 1)],
                                  in_=macc[:, m, :])

        # ---------------- FC head ----------------
        fw1 = plstm.tile([128, 4, 256], bf16, tag="fw1", name="fw1")
        fw2 = plstm.tile([128, 2, 64], bf16, tag="fw2", name="fw2")
        fw3 = plstm.tile([64, 8], bf16, tag="fw3", name="fw3")
        fw4 = plstm.tile([8, 1], bf16, tag="fw4", name="fw4")
        nc.sync.dma_start(out=fw1,
                          in_=din["fw1"].rearrange("(k p) m -> p k m", p=128))
        nc.sync.dma_start(out=fw2,
                          in_=din["fw2"].rearrange("(k p) m -> p k m", p=128))
        nc.sync.dma_start(out=fw3, in_=din["fw3"])
        nc.sync.dma_start(out=fw4, in_=din["fw4"])
        fb1 = pmisc.tile([128, 2], f32)
        fb2 = pmisc.tile([64, 1], f32)
        fb3 = pmisc.tile([8, 1], f32)
        fb4 = pmisc.tile([1, 1], f32)
        nc.sync.dma_start(out=fb1,
                          in_=din["fb1"].rearrange("(j p) -> p j", p=128))
        nc.sync.dma_start(out=fb2, in_=din["fb2"])
        nc.sync.dma_start(out=fb3, in_=din["fb3"])
        nc.sync.dma_start(out=fb4, in_=din["fb4"])

        maccb = prec.tile([128, 4, BC], bf16, tag="maccb", name="maccb")
        nc.vector.tensor_copy(out=maccb, in_=macc)
        z1 = prec.tile([128, 2, BC], bf16, tag="z1", name="z1")
        for mj in range(2):
            ps = psrec.tile([128, BC], f32, tag="ps00", name="psfc")
            for k in range(4):
                nc.tensor.matmul(ps, fw1[:, k, 128 * mj:128 * mj + 128],
                                 maccb[:, k, :], start=(k == 0),
                                 stop=(k == 3))
            nc.scalar.activation(out=z1[:, mj, :], in_=ps, func=AFT.Relu,
                                 bias=fb1[:, mj:mj + 1], scale=1.0)
        z2 = prec.tile([64, BC], bf16, tag="z2", name="z2")
        ps2 = psrec.tile([64, BC], f32, tag="ps00", name="psfc")
        for k in range(2):
            nc.tensor.matmul(ps2, fw2[:, k, 0:64], z1[:, k, :],
                             start=(k == 0), stop=(k == 1))
        nc.scalar.activation(out=z2, in_=ps2, func=AFT.Relu,
                             bias=fb2, scale=1.0)
        z3 = prec.tile([8, BC], bf16, tag="z3", name="z3")
        ps3 = psrec.tile([8, BC], f32, tag="ps00", name="psfc")
        nc.tensor.matmul(ps3, fw3, z2, start=True, stop=True)
        nc.scalar.activation(out=z3, in_=ps3, func=AFT.Relu,
                             bias=fb3, scale=1.0)
        zo = prec.tile([1, BC], f32, tag="zo", name="zo")
        ps4 = psrec.tile([1, BC], f32, tag="ps00", name="psfc")
        nc.tensor.matmul(ps4, fw4, z3, start=True, stop=True)
        nc.scalar.activation(out=zo, in_=ps4, func=AFT.Relu,
                             bias=fb4, scale=1.0)
        nc.sync.dma_start(out=out, in_=zo)


# ---------------------------------------------------------------------------
# host side
# ---------------------------------------------------------------------------

def prep_inputs(x, a, conv1_w, conv1_b, conv2_w, conv2_b, conv3_w, conv3_b,
                W_ih0, W_hh0, b0, W_ih1, W_hh1, b1,
                fc1_w, fc1_b, fc2_w, fc2_b, fc3_w, fc3_b, fc4_w, fc4_b):
    """Returns list of per-core input dicts."""
    p = GATE_PERM
    f = np.float32
    S = ACT_SCALE if FP8 else 1.0
    E4 = mybir.dt.np(f8e4)

    def _whh(w):
        w = np.ascontiguousarray(S * w)
        return w.astype(E4) if FP8 else w.astype(BF)
    shared = {
        "w1": np.ascontiguousarray(np.asarray(conv1_w, f).transpose(1, 2, 0)).astype(BF),
        "w2": np.ascontiguousarray(np.asarray(conv2_w, f).transpose(1, 2, 0)).astype(BF),
        "w3": np.ascontiguousarray(np.asarray(conv3_w, f).transpose(1, 2, 0)).astype(BF),
        "cb1": np.asarray(conv1_b, f), "cb2": np.asarray(conv2_b, f),
        "cb3": np.asarray(conv3_b, f),
        "wih0": np.ascontiguousarray(S * np.asarray(W_ih0, f).T[:, p]).astype(BF),
        "whh0": _whh(np.asarray(W_hh0, f).T[:, p]),
        "bg0": S * np.asarray(b0, f)[p],
        "wih1": np.ascontiguousarray(S * np.asarray(W_ih1, f).T[:, p]).astype(BF),
        "whh1": _whh(np.asarray(W_hh1, f).T[:, p]),
        "bg1": S * np.asarray(b1, f)[p],
        "ident": np.eye(128, dtype=BF),
        "fw1": np.ascontiguousarray(np.asarray(fc1_w, f).T).astype(BF),
        "fw2": np.ascontiguousarray(np.asarray(fc2_w, f).T).astype(BF),
        "fw3": np.ascontiguousarray(np.asarray(fc3_w, f).T).astype(BF),
        "fw4": np.ascontiguousarray(np.asarray(fc4_w, f).T).astype(BF),
        "fb1": np.asarray(fc1_b, f), "fb2": np.asarray(fc2_b, f),
        "fb3": np.asarray(fc3_b, f), "fb4": np.asarray(fc4_b, f),
    }
    x = np.asarray(x, f)
    a = np.asarray(a)
    in_maps = []
    for c in range(N_CORES):
        xs = x[BC * c:BC * (c + 1)]             # [BC, CIN, T]
        xbp = np.zeros((CIN, BC, T + 2), BF)
        xbp[:, :, 1:T + 1] = xs.transpose(1, 0, 2).astype(BF)
        ash = a[BC * c:BC * (c + 1)].astype(np.int64)
        mkv = (np.arange(T2)[:, None] < ash[None, :]).astype(f)
        mkv = mkv / ash[None, :].astype(f)
        m = dict(shared)
        m["xb"] = xbp
        m["mk"] = np.ascontiguousarray(mkv.T)
        in_maps.append(m)
    return in_maps


_CACHED_NC = None


def kernel(**inputs):
    global _CACHED_NC
    if _CACHED_NC is None:
        _CACHED_NC = build_kernel()
    in_maps = prep_inputs(**inputs)
    res = run_bass_kernel_spmd(_CACHED_NC, in_maps,
                               core_ids=list(range(N_CORES)))
    z = np.concatenate([res.results[c]["out"] for c in range(N_CORES)])
    return z.reshape(B, 1).astype(np.float32)

